# revision 36
# baseline (speedup 1.0000x reference)
"""Trainium2 Bass kernel for a 6-layer post-BatchNorm transformer encoder.

Reference model:
  x = emb[seq] + pes                                  # [B,S,D] = [4,512,1024]
  6x: x = BN(x + attn(x)); x = BN(x + ffn(x))
  BN = per-channel batch stats over (B,S), eps=1e-3.

Sharding: token-sharded data parallel across 8 NeuronCores. Core c owns the
256 contiguous tokens [256c, 256c+256) = batch c//2, sequence half c%2. All
matmuls are local full-width (every core streams the full bf16 weights from
HBM in 1MB chunks); residual adds and BatchNorm application are local.

Cross-core communication per layer:
  - one pair AllGather (cores 2b,2b+1) of the raw pre-BN2 activations y2
    (bf16, 0.5MB in / 1MB out) fired concurrently with the BN2 stats
    collective; BN2's apply is a per-channel affine with globally
    identical sc/sh, so each core applies it in place to the landed
    buffer to assemble xext (the batch's full 512 post-BN tokens). Each
    core then recomputes K and V for all 512 tokens locally. Attention is
    invariant to key order, so the rank-major half order in xext needs no
    per-core (rank-dependent) indexing anywhere. Q projection (local
    tokens) covers the collective tail.
  - two 8KB AllGathers of per-core BN partial sums/sumsq (the only global
    coupling BatchNorm actually needs).
bk/bv/bo/b2 biases cancel mathematically (bk/bv through softmax rows summing
to 1, bo/b2 inside BN mean subtraction) and are dropped; bq and b1 are kept.

Scheduling notes:
  - The softmax normalization reciprocals are broadcast into the dead
    partition halves of the two U PSUM banks (no third PSUM tile).
  - Next layer's Wq/Wk/Wv chunks are requested after FFN1 on the gpsimd
    DMA queue so they never sit behind W1/W2 sync-queue traffic.
  - Dummy keep-warm matmuls (ones x ones into a scratch PSUM bank) are
    issued into each BatchNorm sync bubble: they keep the PE activity
    monitor from re-throttling the clock to 1.2GHz after every
    collective (the re-warm otherwise costs ~30us at half clock).
  - BN rstd uses DVE reciprocal + ACT sqrt; the sqrt<->exp table-set
    reloads hide behind the stats landing DMA / attention score matmuls.

Numerics: all matmuls in bf16 (fp32 PSUM accumulation); x kept in fp32
master + bf16 matmul copy; BN statistics in fp32.

Layout: activations transposed [128 part, dtile, tokens]; weights natural
[Din, Dout] serve as lhsT. Attention per head pair packs the two heads at
partition bases 0/64 (row-group concurrency on the PE). Softmax sums ride
as a ones-column in the even head's V (PSUM rows 64) and a ones-row matmul
into PSUM row 96 for the odd head. Embedding gather uses
dma_gather(transpose=True), which lands rows directly in the transposed
layout.
"""

import os

import numpy as np

import concourse.bass as bass
import concourse.mybir as mybir
import concourse.tile as tile
from concourse import bacc
from concourse.bass import ts

# ---------------------------------------------------------------- dims
V, D, L, H, B, S = 32000, 1024, 6, 16, 4, 512
HD = D // H            # 64
DF = 4 * D             # 4096
EPS = 1e-3
NC = 8                 # cores
T = B * S              # 2048 tokens total
TL = T // NC           # 256 tokens per core
P = 128                # partitions
DT = D // P            # 8 d-tiles
FMT = DF // P          # 32 ffn1 m-tiles
SK = S // P            # 4 key chunks per batch

f32 = mybir.dt.float32
bf16 = mybir.dt.bfloat16
i16 = mybir.dt.int16
AF = mybir.ActivationFunctionType
ALU = mybir.AluOpType

ALLGRP = [list(range(NC))]
KVGRP = [[2 * b, 2 * b + 1] for b in range(B)]

N_LAYERS = int(os.environ.get("TRN_KERNEL_LAYERS", str(L)))
DEBUG_TAPS = os.environ.get("TRN_KERNEL_DEBUG", "0") == "1"
DBG_LAYER = int(os.environ.get("TRN_DBG_LAYER", "0"))

XB = D * TL            # elements of one rank's xb16 block in the x exchange


def build_module(n_layers=None):
    if n_layers is None:
        n_layers = N_LAYERS
    nc = bacc.Bacc("TRN2", target_bir_lowering=False, debug=False,
                   num_devices=NC)

    dt_ = nc.dram_tensor
    io = {
        "emb": dt_("emb", [V, D], bf16, kind="ExternalInput").ap(),
        "idx": dt_("idx", [16, TL // 16], i16, kind="ExternalInput").ap(),
        "pesT": dt_("pesT", [D, TL], f32, kind="ExternalInput").ap(),
        "wq": dt_("wq", [L, D, D], bf16, kind="ExternalInput").ap(),
        "wk": dt_("wk", [L, D, D], bf16, kind="ExternalInput").ap(),
        "wv": dt_("wv", [L, D, D], bf16, kind="ExternalInput").ap(),
        "wo": dt_("wo", [L, D, D], bf16, kind="ExternalInput").ap(),
        "w1": dt_("w1", [L, D, DF], bf16, kind="ExternalInput").ap(),
        "w2": dt_("w2", [L, DF, D], bf16, kind="ExternalInput").ap(),
        "bq": dt_("bq", [L, P, DT], f32, kind="ExternalInput").ap(),
        "b1": dt_("b1", [L, P, FMT], f32, kind="ExternalInput").ap(),
        "g1": dt_("g1", [L, P, DT], f32, kind="ExternalInput").ap(),
        "be1": dt_("be1", [L, P, DT], f32, kind="ExternalInput").ap(),
        "g2": dt_("g2", [L, P, DT], f32, kind="ExternalInput").ap(),
        "be2": dt_("be2", [L, P, DT], f32, kind="ExternalInput").ap(),
        "out": dt_("out", [D, TL], f32, kind="ExternalOutput").ap(),
    }
    if DEBUG_TAPS:
        for nm, shp, dt in [
                ("dbg_xext", [D, S], bf16), ("dbg_q", [D, TL], bf16),
                ("dbg_k", [D, S], bf16),
                ("dbg_v", [P, SK * H * (HD + 1)], bf16),
                ("dbg_attnT", [D, TL], bf16), ("dbg_y1", [D, TL], f32),
                ("dbg_x2", [D, TL], f32), ("dbg_h", [DF, TL], bf16),
                ("dbg_y2", [D, TL], f32)]:
            io[nm] = dt_(nm, shp, dt, kind="ExternalOutput").ap()

    with tile.TileContext(nc) as tc:
        _build(tc, n_layers, io)
    nc.compile()
    return nc


def _build(tc, n_layers, io):
    from contextlib import ExitStack
    nc = tc.nc
    att_scale = 1.0 / np.sqrt(HD)
    dmae = nc.scalar

    st_ = ExitStack()
    persist = st_.enter_context(tc.tile_pool(name="persist", bufs=1))
    wpool = st_.enter_context(tc.tile_pool(name="wpool", bufs=10))
    ppool = st_.enter_context(tc.tile_pool(name="ppool", bufs=2))
    epool = st_.enter_context(tc.tile_pool(name="epool", bufs=10))
    spool = st_.enter_context(tc.tile_pool(name="spool", bufs=2))
    ps = st_.enter_context(tc.tile_pool(name="ps", bufs=4, space="PSUM"))
    drin = st_.enter_context(tc.tile_pool(name="drin", bufs=2, space="DRAM"))
    drout = st_.enter_context(tc.tile_pool(name="drout", bufs=2, space="DRAM"))

    # ---------------- persistent tiles
    xf32a = persist.tile([P, DT, TL], f32, name="xf32a")
    xf32b = persist.tile([P, DT, TL], f32, name="xf32b")
    xb16 = persist.tile([P, DT, TL], bf16, name="xb16")
    xext = persist.tile([P, DT, S], bf16, name="xext")
    qT = persist.tile([P, DT, TL], bf16, name="qT")
    kT = persist.tile([P, DT, S], bf16, name="kT")
    vsb = persist.tile([P, SK, H, HD + 1], bf16, name="vsb")
    attnT = persist.tile([P, DT, TL], bf16, name="attnT")
    ht = persist.tile([P, FMT, TL], bf16, name="ht")
    y2b16 = persist.tile([P, DT, TL], bf16, name="y2b16")
    onesb = persist.tile([P, P], bf16, name="onesb")
    dmx = persist.tile([P, 512], bf16, name="dmx")
    dscr = persist.tile([P, 1], bf16, name="dscr")
    epsb = persist.tile([P, 1], f32, name="epsb")
    idxs = persist.tile([P, TL // 16], i16, name="idxs")

    nc.vector.memset(onesb[:], 1.0)
    nc.vector.memset(dmx[:], 1.0)
    nc.vector.memset(epsb[:], EPS)
    nc.vector.memset(vsb[:, :, :, HD:HD + 1], 1.0)
    for r_ in range(P // 16):
        nc.sync.dma_start(idxs[16 * r_:16 * (r_ + 1), :], io["idx"])

    # weight chunk loader: returns [P, DT, 512] (half the out-cols of a DxD
    # weight) or [P, 4, D] (4 k-tiles of w2). Q/K/V chunks ride the gpsimd
    # engine's DMA queue so they never sit behind the 16MB of W1/W2 traffic
    # on the sync queue (Q proj must start the moment BN2's xb16 lands).
    def wchunk(src_ap, nm, eng=None):
        t = wpool.tile(list(src_ap.shape), bf16, tag="w", name=nm)
        (eng or nc.sync).dma_start(t[:], src_ap)
        return t

    def load_kvq(l):
        wq_r = io["wq"][l].rearrange("(k p) m -> p k m", p=P)
        wk_r = io["wk"][l].rearrange("(k p) m -> p k m", p=P)
        wv_r = io["wv"][l].rearrange("(k p) m -> p k m", p=P)
        wq_ch = [wchunk(wq_r[:, :, ts(h, 512)], f"wq{l}_{h}", nc.gpsimd)
                 for h in range(2)]
        wk_ch = [wchunk(wk_r[:, :, ts(h, 512)], f"wk{l}_{h}", nc.gpsimd)
                 for h in range(2)]
        wv_ch = [wchunk(wv_r[:, :, ts(h, 512)], f"wv{l}_{h}", nc.gpsimd)
                 for h in range(2)]
        return wq_ch, wk_ch, wv_ch

    # keep-warm fillers: dense dummy matmuls issued into a sync bubble so
    # the PE activity monitor doesn't re-throttle the clock. They read only
    # never-rewritten tiles (onesb/dmx) so they create no dependencies and
    # start the moment the preceding real matmul retires.
    NFILL = int(os.environ.get("TRN_FILL", "32"))
    NFILLG = int(os.environ.get("TRN_FILLG", "0"))

    def pe_fill(nm, anchor, n=None):
        n = NFILL if n is None else n
        if n == 0:
            return
        scr = ps.tile([P, 512], f32, tag="u", bufs=2, name=f"fill{nm}")
        nc.tensor.matmul(scr[:], onesb[:], anchor, start=True, stop=True)
        for _ in range(n - 1):
            nc.tensor.matmul(scr[:], onesb[:], dmx[:], start=True, stop=True)
        with nc.allow_low_precision(reason="keep-warm filler sink"):
            nc.vector.tensor_copy(dscr[:, 0:1], scr[:, 0:1])

    # pair AllGather of src (xb16 at the embedding; raw y2 per layer): both
    # halves land in xext in fixed rank-major order. Attention is key-order
    # invariant, so no rank-dependent indexing is needed anywhere.
    def x_ship(lbl, src):
        xi = drin.tile([XB], bf16, tag="xi", name=f"xi{lbl}")
        xo = drout.tile([2 * XB], bf16, tag="xo", name=f"xo{lbl}")
        dmae.dma_start(
            xi[:].rearrange("(g p t) -> p g t", g=DT, p=P), src[:])
        nc.gpsimd.collective_compute(
            "AllGather", ALU.bypass, replica_groups=KVGRP,
            ins=[xi[:].opt()], outs=[xo[:].opt()])
        return xo

    def x_land(xo, sc=None, sh=None):
        for hf in range(2):
            dmae.dma_start(
                xext[:, :, ts(hf, TL)],
                xo[hf * XB:(hf + 1) * XB].rearrange(
                    "(g p t) -> p g t", g=DT, p=P))
        if sc is not None:
            for m in range(DT):
                nc.vector.tensor_scalar(out=xext[:, m, :], in0=xext[:, m, :],
                                        scalar1=sc[:, m:m + 1],
                                        scalar2=sh[:, m:m + 1],
                                        op0=ALU.mult, op1=ALU.add)

    # ---------------- embedding: x^T = (emb[seq])^T + pes^T
    # (gather first: layer-0's kvq prefetch shares the gpsimd DMA queue and
    # must not delay the critical path to xb16 / the first x exchange)
    pes_sb = spool.tile([P, DT, TL], f32, tag="pes", bufs=1, name="pes_sb")
    dmae.dma_start(pes_sb[:], io["pesT"].rearrange("(k p) t -> p k t", p=P))
    gt = spool.tile([P, DT, TL], bf16, tag="gt", bufs=1, name="gt")
    nc.gpsimd.dma_gather(
        out_ap=gt[:], in_ap=io["emb"], idxs_ap=idxs[:],
        num_idxs=TL, num_idxs_reg=TL, elem_size=D, transpose=True)
    kvq_next = load_kvq(0)
    for k in range(DT):
        nc.vector.tensor_tensor(out=xf32a[:, k, :], in0=gt[:, k, :],
                                in1=pes_sb[:, k, :], op=ALU.add)
    nc.vector.tensor_copy(xb16[:], xf32a[:])
    x_land(x_ship("e", xb16))

    xcur = xf32a
    xnxt = xf32b

    # ---------------- per-layer param loads (small)
    def load_params(l):
        bq_sb = ppool.tile([P, DT], f32, tag="bq", name=f"bq{l}")
        b1_sb = ppool.tile([P, FMT], f32, tag="b1", name=f"b1{l}")
        g1_sb = ppool.tile([P, DT], f32, tag="g1", name=f"g1{l}")
        be1_sb = ppool.tile([P, DT], f32, tag="be1", name=f"be1{l}")
        g2_sb = ppool.tile([P, DT], f32, tag="g2", name=f"g2{l}")
        be2_sb = ppool.tile([P, DT], f32, tag="be2", name=f"be2{l}")
        dmae.dma_start(bq_sb[:], io["bq"][l])
        dmae.dma_start(b1_sb[:], io["b1"][l])
        dmae.dma_start(g1_sb[:], io["g1"][l])
        dmae.dma_start(be1_sb[:], io["be1"][l])
        dmae.dma_start(g2_sb[:], io["g2"][l])
        dmae.dma_start(be2_sb[:], io["be2"][l])
        return bq_sb, b1_sb, g1_sb, be1_sb, g2_sb, be2_sb

    # BN stats -> AllGather -> sc/sh. rstd = exp(-0.5*ln(var+eps)) keeps the
    # ACT engine on the natural_log_exp table set for the whole kernel (the
    # Sqrt set would force a ~2.7us table reload twice per layer).
    def bn_reduce(lbl, stats, g_sb, be_sb):
        sti = drin.tile([P * 16], f32, tag="sti", name=f"sti{lbl}")
        sto = drout.tile([NC * P * 16], f32, tag="sto", addr_space="Shared",
                         name=f"sto{lbl}")
        dmae.dma_start(sti[:].rearrange("(p s) -> p s", p=P), stats[:])
        nc.gpsimd.collective_compute(
            "AllGather", ALU.bypass, replica_groups=ALLGRP,
            ins=[sti[:].opt()], outs=[sto[:].opt()])
        # land rank-major (16 contiguous fp32 per descriptor segment; an
        # s-innermost layout would degrade the DMA to 4-byte elements and
        # cost ~20us), then tree-reduce the 8 rank blocks on the DVE.
        ld = spool.tile([P, NC, 16], f32, tag="ld", name=f"ld{lbl}")
        dmae.dma_start(ld[:], sto[:].rearrange("(r p s) -> p r s", p=P, s=16))
        u1 = spool.tile([P, 4, 16], f32, tag="u1", name=f"u1{lbl}")
        nc.vector.tensor_tensor(out=u1[:], in0=ld[:, 0:4, :], in1=ld[:, 4:8, :],
                                op=ALU.add)
        u2 = spool.tile([P, 2, 16], f32, tag="u2", name=f"u2{lbl}")
        nc.vector.tensor_tensor(out=u2[:], in0=u1[:, 0:2, :], in1=u1[:, 2:4, :],
                                op=ALU.add)
        tot = spool.tile([P, 16], f32, tag="tot", name=f"tot{lbl}")
        nc.vector.tensor_tensor(out=tot[:], in0=u2[:, 0, :], in1=u2[:, 1, :],
                                op=ALU.add)
        mean = spool.tile([P, DT], f32, tag="mean", name=f"mean{lbl}")
        nc.vector.tensor_scalar_mul(mean[:], tot[:, 0:DT], 1.0 / T)
        msq = spool.tile([P, DT], f32, tag="msq", name=f"msq{lbl}")
        nc.vector.tensor_tensor(out=msq[:], in0=mean[:], in1=mean[:], op=ALU.mult)
        veps = spool.tile([P, DT], f32, tag="veps", name=f"veps{lbl}")
        nc.vector.scalar_tensor_tensor(out=veps[:], in0=tot[:, DT:16],
                                       scalar=1.0 / T, in1=msq[:],
                                       op0=ALU.mult, op1=ALU.subtract)
        if os.environ.get("TRN_RSTD_SQRT", "1") == "1":
            nc.vector.tensor_scalar_add(veps[:], veps[:], EPS)
            rec = spool.tile([P, DT], f32, tag="lnv", name=f"rec{lbl}")
            nc.vector.reciprocal(rec[:], veps[:])
            rstd = spool.tile([P, DT], f32, tag="rstd", name=f"rstd{lbl}")
            nc.scalar.sqrt(rstd[:], rec[:])
        else:
            lnv = spool.tile([P, DT], f32, tag="lnv", name=f"lnv{lbl}")
            nc.scalar.activation(lnv[:], veps[:], AF.Ln, bias=epsb[:, 0:1])
            rstd = spool.tile([P, DT], f32, tag="rstd", name=f"rstd{lbl}")
            nc.scalar.activation(rstd[:], lnv[:], AF.Exp, scale=-0.5)
        sc = spool.tile([P, DT], f32, tag="sc", name=f"sc{lbl}")
        nc.vector.tensor_tensor(out=sc[:], in0=g_sb[:], in1=rstd[:], op=ALU.mult)
        sh = spool.tile([P, DT], f32, tag="sh", name=f"sh{lbl}")
        nc.vector.tensor_tensor(out=sh[:], in0=mean[:], in1=sc[:], op=ALU.mult)
        nc.vector.tensor_tensor(out=sh[:], in0=be_sb[:], in1=sh[:], op=ALU.subtract)
        return sc, sh

    # ---------------- layers
    for l in range(n_layers):
        bq_sb, b1_sb, g1_sb, be1_sb, g2_sb, be2_sb = load_params(l)
        wq_ch, wk_ch, wv_ch = kvq_next

        wo_r = io["wo"][l].rearrange("(k p) m -> p k m", p=P)
        w1_r = io["w1"][l].rearrange("(k p) m -> p k m", p=P)
        w2_r = io["w2"][l].rearrange("(k p) m -> p k m", p=P)

        # ---- Q projection (local tokens; overlaps the x AllGather)
        for g in range(DT):
            psq = ps.tile([P, TL], f32, tag="mm", name=f"psq{l}_{g}")
            for k in range(DT):
                nc.tensor.matmul(psq[:], wq_ch[g // 4][:, k, ts(g % 4, P)],
                                 xb16[:, k, :], start=(k == 0), stop=(k == DT - 1))
            nc.vector.tensor_scalar_add(qT[:, g, :], psq[:], bq_sb[:, g:g + 1])

        # keep the clock warm across the x-AllGather tail (K/V proj below
        # block on the xext landing; Q proj above is the only real cover)
        pe_fill(f"q{l}", qT[:, 6:8, :], n=40)

        # ---- K projection over the full 512-token batch sequence
        for g in range(DT):
            psk = ps.tile([P, S], f32, tag="mm", name=f"psk{l}_{g}")
            for k in range(DT):
                nc.tensor.matmul(psk[:], wk_ch[g // 4][:, k, ts(g % 4, P)],
                                 xext[:, k, :], start=(k == 0), stop=(k == DT - 1))
            nc.vector.tensor_copy(kT[:, g, :], psk[:])

        # ---- V projection, token-major: V = x W_v (xext tiles stationary)
        for mt in range(SK):
            for nb in range(4):
                psv = ps.tile([P, TL], f32, tag="mm", name=f"psv{l}_{mt}_{nb}")
                for k in range(DT):
                    nc.tensor.matmul(
                        psv[:], xext[:, k, ts(mt, P)],
                        wv_ch[nb // 2][:, k, ts(nb % 2, 256)],
                        start=(k == 0), stop=(k == DT - 1))
                nc.vector.tensor_copy(
                    vsb[:, mt, 4 * nb:4 * nb + 4, 0:HD],
                    psv[:].rearrange("p (h x) -> p h x", h=4))

        if DEBUG_TAPS and l == DBG_LAYER:
            nc.sync.dma_start(io["dbg_xext"].rearrange("(k p) t -> p k t", p=P),
                              xext[:])
            nc.sync.dma_start(io["dbg_q"].rearrange("(k p) t -> p k t", p=P),
                              qT[:])
            nc.sync.dma_start(io["dbg_k"].rearrange("(k p) t -> p k t", p=P),
                              kT[:])
            nc.sync.dma_start(io["dbg_v"],
                              vsb[:].rearrange("p a h x -> p (a h x)"))

        wo_ch = [wchunk(wo_r[:, :, ts(h, 512)], f"wo{l}_{h}") for h in range(2)]

        # ---- attention phase A: scores + exp (batched over 2 key-chunks)
        eall = []
        for g in range(DT):
            epair = [[None, None], [None, None]]
            for kcb in range(2):
                for tw in range(2):
                    hp = 64 * tw
                    sst = ps.tile([P, 2, TL], f32, tag="s", bufs=2,
                                  name=f"pss{l}_{g}_{kcb}_{tw}")
                    for j in range(2):
                        nc.tensor.matmul(
                            sst[:, j, :],
                            kT[hp:hp + HD, g, ts(2 * kcb + j, P)],
                            qT[hp:hp + HD, g, :], start=True, stop=True)
                    et = epool.tile([P, 2, TL], bf16, tag="e", bufs=16,
                                    name=f"et{l}_{g}_{kcb}_{tw}")
                    nc.scalar.activation(et[:], sst[:], AF.Exp, scale=att_scale)
                    epair[tw][kcb] = et
            eall.append(epair)

        # ---- phase B + k-outer Wo: per pair g accumulate U over the 4 key
        # chunks, normalize, then feed the Wo accumulators with attnT[:,g,:]
        # to keep the PE dense through the ACT-bound softmax phase.
        wo_acc = [ps.tile([P, 2, TL], f32, tag="mm", name=f"woa{l}_{m}")
                  for m in range(4)]
        st1 = spool.tile([P, 16], f32, tag="st", name=f"st1_{l}")
        sqs = spool.tile([P, TL], f32, tag="sqs", name=f"sq1_{l}")
        for g in range(DT):
            epair = eall[g]
            bankA = ps.tile([P, TL], f32, tag="u", bufs=2, name=f"bA{l}_{g}")
            bankB = ps.tile([P, TL], f32, tag="u", bufs=2, name=f"bB{l}_{g}")
            for kc in range(SK):
                fl, ll = (kc == 0), (kc == SK - 1)
                ee = epair[0][kc // 2][:, kc % 2, :]
                eo = epair[1][kc // 2][:, kc % 2, :]
                nc.tensor.matmul(bankA[0:HD + 1, :],
                                 vsb[:, kc, 2 * g, 0:HD + 1], ee,
                                 start=fl, stop=ll)
                nc.tensor.matmul(bankB[64:128, :],
                                 vsb[:, kc, 2 * g + 1, 0:HD], eo,
                                 start=fl, stop=ll)
                nc.tensor.matmul(bankA[96:97, :], onesb[:, 0:1], eo,
                                 start=fl, stop=ll, tile_position=(0, 96))
            rs = spool.tile([P, TL], bf16, tag="rs", name=f"rs{l}_{g}")
            with nc.allow_low_precision(reason="softmax 1/sumexp as bf16"):
                nc.vector.reciprocal(rs[HD:97, :], bankA[HD:97, :])
            # broadcast the reciprocals into the dead halves of the U banks
            nc.tensor.matmul(bankB[0:64, :], onesb[HD:HD + 1, 0:64],
                             rs[HD:HD + 1, :], start=True, stop=True,
                             tile_position=(64, 0))
            nc.tensor.matmul(bankA[64:128, :], onesb[96:97, 0:64],
                             rs[96:97, :], start=True, stop=True,
                             tile_position=(96, 64))
            usbE = epool.tile([P, TL], bf16, tag="usb", bufs=6,
                              name=f"uE{l}_{g}")
            usbO = epool.tile([P, TL], bf16, tag="usb", bufs=6,
                              name=f"uO{l}_{g}")
            nc.vector.tensor_copy(usbE[0:64, :], bankA[0:64, :])
            nc.vector.tensor_copy(usbO[64:128, :], bankB[64:128, :])
            nc.vector.tensor_tensor(out=attnT[0:64, g, :], in0=usbE[0:64, :],
                                    in1=bankB[0:64, :], op=ALU.mult)
            nc.vector.tensor_tensor(out=attnT[64:128, g, :], in0=usbO[64:128, :],
                                    in1=bankA[64:128, :], op=ALU.mult)
            if NFILLG and g < DT - 1:
                # keep the PE activity monitor fed through the exp-bound
                # stretch between head pairs (attention otherwise cools the
                # clock to 1.2GHz for the rest of the phase)
                scrg = ps.tile([P, 512], f32, tag="u", bufs=2,
                               name=f"fillg{l}_{g}")
                nc.tensor.matmul(scrg[:, 0:TL], onesb[:], attnT[:, g, :],
                                 start=True, stop=True)
                for _ in range(NFILLG - 1):
                    nc.tensor.matmul(scrg[:], onesb[:], dmx[:],
                                     start=True, stop=True)
                with nc.allow_low_precision(reason="keep-warm filler sink"):
                    nc.vector.tensor_copy(dscr[:, 0:1], scrg[:, 0:1])

        for m in range(DT):
            for g in range(DT):
                nc.tensor.matmul(wo_acc[m // 2][:, m % 2, :],
                                 wo_ch[m // 4][:, g, ts(m % 4, P)],
                                 attnT[:, g, :],
                                 start=(g == 0), stop=(g == DT - 1))

        w1_ch = [wchunk(w1_r[:, :, ts(h, 512)], f"w1{l}_{h}") for h in range(8)]

        # ---- residual -> y1 (fp32) with fused BN partial stats
        for m in range(DT):
            nc.vector.scalar_tensor_tensor(
                out=xnxt[:, m, :], in0=wo_acc[m // 2][:, m % 2, :], scalar=1.0,
                in1=xcur[:, m, :], op0=ALU.mult, op1=ALU.add,
                accum_out=st1[:, m:m + 1])
            nc.scalar.activation(sqs[:], xnxt[:, m, :], AF.Square,
                                 accum_out=st1[:, DT + m:DT + m + 1])

        if DEBUG_TAPS and l == DBG_LAYER:
            nc.sync.dma_start(io["dbg_attnT"].rearrange("(k p) t -> p k t", p=P),
                              attnT[:])
            nc.sync.dma_start(io["dbg_y1"].rearrange("(k p) t -> p k t", p=P),
                              xnxt[:])

        pe_fill(f"a{l}", attnT[:, 6:8, :])
        sc1, sh1 = bn_reduce(f"a{l}", st1, g1_sb, be1_sb)
        for m in range(DT):
            nc.scalar.activation(xb16[:, m, :], xnxt[:, m, :], AF.Identity,
                                 bias=sh1[:, m:m + 1], scale=sc1[:, m:m + 1])
            nc.vector.tensor_scalar(out=xnxt[:, m, :], in0=xnxt[:, m, :],
                                    scalar1=sc1[:, m:m + 1],
                                    scalar2=sh1[:, m:m + 1],
                                    op0=ALU.mult, op1=ALU.add)
        xcur, xnxt = xnxt, xcur

        if DEBUG_TAPS and l == DBG_LAYER:
            nc.sync.dma_start(io["dbg_x2"].rearrange("(k p) t -> p k t", p=P),
                              xcur[:])

        w2_ch = [wchunk(w2_r[:, ts(h, 4), :], f"w2{l}_{h}") for h in range(8)]

        # ---- FFN1: h^T = relu(W1^T x^T + b1)
        for m in range(FMT):
            ps1 = ps.tile([P, TL], f32, tag="mm", name=f"ps1{l}_{m}")
            for k in range(DT):
                nc.tensor.matmul(ps1[:], w1_ch[m // 4][:, k, ts(m % 4, P)],
                                 xb16[:, k, :], start=(k == 0), stop=(k == DT - 1))
            nc.scalar.activation(ht[:, m, :], ps1[:], AF.Relu,
                                 bias=b1_sb[:, m:m + 1])

        # ---- FFN2 + residual -> y2 with fused BN partial stats
        st2 = spool.tile([P, 16], f32, tag="st", name=f"st2_{l}")
        sqs2 = spool.tile([P, TL], f32, tag="sqs", name=f"sq2_{l}")
        HF = FMT // 2
        for m in range(DT):
            ps2 = ps.tile([P, TL], f32, tag="mm", name=f"ps2a{l}_{m}")
            for k in range(HF):
                nc.tensor.matmul(ps2[:], w2_ch[k // 4][:, k % 4, ts(m, P)],
                                 ht[:, k, :], start=(k == 0), stop=(k == HF - 1))
            nc.vector.scalar_tensor_tensor(
                out=xnxt[:, m, :], in0=ps2[:], scalar=1.0, in1=xcur[:, m, :],
                op0=ALU.mult, op1=ALU.add)
        for m in range(DT):
            ps2 = ps.tile([P, TL], f32, tag="mm", name=f"ps2b{l}_{m}")
            for k in range(HF, FMT):
                nc.tensor.matmul(ps2[:], w2_ch[k // 4][:, k % 4, ts(m, P)],
                                 ht[:, k, :], start=(k == HF), stop=(k == FMT - 1))
            nc.vector.scalar_tensor_tensor(
                out=xnxt[:, m, :], in0=ps2[:], scalar=1.0, in1=xnxt[:, m, :],
                op0=ALU.mult, op1=ALU.add, accum_out=st2[:, m:m + 1])
            nc.scalar.activation(sqs2[:], xnxt[:, m, :], AF.Square,
                                 accum_out=st2[:, DT + m:DT + m + 1])
            if l + 1 < n_layers:
                nc.scalar.copy(y2b16[:, m, :], xnxt[:, m, :])

        if DEBUG_TAPS and l == DBG_LAYER:
            nc.sync.dma_start(io["dbg_h"].rearrange("(k p) t -> p k t", p=P),
                              ht[:])
            nc.sync.dma_start(io["dbg_y2"].rearrange("(k p) t -> p k t", p=P),
                              xnxt[:])

        if l + 1 < n_layers:
            pe_fill(f"f{l}", y2b16[:, 6:8, :], n=44)
        sc2, sh2 = bn_reduce(f"f{l}", st2, g2_sb, be2_sb)
        if l + 1 < n_layers:
            xo_next = x_ship(f"x{l + 1}", y2b16)
            x_land(xo_next, sc2, sh2)
            # prefetch next layer's Q/K/V weights into the BN2 bubble (the
            # gpsimd DMA queue is otherwise idle here, and this keeps their
            # 6MB off the HBM while FFN2 streams W2)
            kvq_next = load_kvq(l + 1)
        for m in range(DT):
            if l + 1 < n_layers:
                nc.scalar.activation(xb16[:, m, :], xnxt[:, m, :], AF.Identity,
                                     bias=sh2[:, m:m + 1], scale=sc2[:, m:m + 1])
            nc.vector.tensor_scalar(out=xnxt[:, m, :], in0=xnxt[:, m, :],
                                    scalar1=sc2[:, m:m + 1],
                                    scalar2=sh2[:, m:m + 1],
                                    op0=ALU.mult, op1=ALU.add)
        xcur, xnxt = xnxt, xcur

    # ---------------- output x^T local slice (per tile, overlapping the
    # final BN apply chain)
    outr = io["out"].rearrange("(k p) t -> p k t", p=P)
    for m in range(DT):
        dmae.dma_start(outr[:, m, :], xcur[:, m, :])
    st_.close()


# ================================================================ host side

def make_in_maps(inputs):
    import ml_dtypes
    bf = lambda a: np.ascontiguousarray(np.asarray(a, dtype=np.float32)).astype(
        ml_dtypes.bfloat16)
    f = lambda a: np.ascontiguousarray(np.asarray(a), dtype=np.float32)
    seq = np.asarray(inputs["sequence"]).reshape(-1).astype(np.int16)
    emb = bf(inputs["emb"])
    pesT = np.ascontiguousarray(f(inputs["pes"]).T)            # [D, S]
    wq, wk, wv = bf(inputs["Wq"]), bf(inputs["Wk"]), bf(inputs["Wv"])
    wo, w1, w2 = bf(inputs["Wo"]), bf(inputs["W1"]), bf(inputs["W2"])
    pt = lambda a, m: np.ascontiguousarray(
        f(a).reshape(L, m, P).transpose(0, 2, 1))   # [L, P, m] with ch = m*128+p
    bq, b1 = pt(inputs["bq"], DT), pt(inputs["b1"], FMT)
    g1, be1 = pt(inputs["g1"], DT), pt(inputs["be1"], DT)
    g2, be2 = pt(inputs["g2"], DT), pt(inputs["be2"], DT)

    in_maps = []
    for c in range(NC):
        loc = seq[c * TL:(c + 1) * TL]
        idx = np.ascontiguousarray(loc.reshape(TL // 16, 16).T)    # [16, TL/16]
        off = (c % 2) * TL
        in_maps.append({
            "emb": emb,
            "idx": idx,
            "pesT": np.ascontiguousarray(pesT[:, off:off + TL]),
            "wq": wq, "wk": wk, "wv": wv, "wo": wo, "w1": w1, "w2": w2,
            "bq": bq, "b1": b1,
            "g1": g1, "be1": be1, "g2": g2, "be2": be2,
        })
    return in_maps


_CACHE = {}


def _get_module():
    if "nc" not in _CACHE:
        _CACHE["nc"] = build_module()
    return _CACHE["nc"]


def kernel(**inputs):
    from concourse import bass_utils
    nc = _get_module()
    in_maps = make_in_maps(inputs)
    res = bass_utils.run_bass_kernel_spmd(nc, in_maps, list(range(NC)))
    full = np.concatenate(
        [np.asarray(res.results[c]["out"]) for c in range(NC)], axis=1)
    return np.ascontiguousarray(full.T).reshape(B, S, D).astype(np.float32)


# revision 38
# speedup vs baseline: 1.0556x; 1.0556x over previous
"""Trainium2 Bass kernel for a 6-layer post-BatchNorm transformer encoder.

Reference model:
  x = emb[seq] + pes                                  # [B,S,D] = [4,512,1024]
  6x: x = BN(x + attn(x)); x = BN(x + ffn(x))
  BN = per-channel batch stats over (B,S), eps=1e-3.

Sharding: token-sharded data parallel across 8 NeuronCores. Core c owns the
256 contiguous tokens [256c, 256c+256) = batch c//2, sequence half c%2. All
matmuls are local full-width (every core streams the full bf16 weights from
HBM in 1MB chunks); residual adds and BatchNorm application are local.

Cross-core communication per layer:
  - one pair AllGather (cores 2b,2b+1) of the raw pre-BN2 activations y2
    (bf16, 0.5MB in / 1MB out) fired concurrently with the BN2 stats
    collective; BN2's apply is a per-channel affine with globally
    identical sc/sh, so each core applies it in place to the landed
    buffer to assemble xext (the batch's full 512 post-BN tokens). Each
    core then recomputes K and V for all 512 tokens locally. Attention is
    invariant to key order, so the rank-major half order in xext needs no
    per-core (rank-dependent) indexing anywhere. Q projection (local
    tokens) covers the collective tail.
  - two 8KB AllGathers of per-core BN partial sums/sumsq (the only global
    coupling BatchNorm actually needs).
bk/bv/bo/b2 biases cancel mathematically (bk/bv through softmax rows summing
to 1, bo/b2 inside BN mean subtraction) and are dropped; bq and b1 are kept.

Scheduling notes:
  - The softmax normalization reciprocals are broadcast into the dead
    partition halves of the two U PSUM banks (no third PSUM tile).
  - Next layer's Wq/Wk/Wv chunks are requested after FFN1 on the gpsimd
    DMA queue so they never sit behind W1/W2 sync-queue traffic.
  - Dummy keep-warm matmuls (ones x ones into a scratch PSUM bank) are
    issued into each BatchNorm sync bubble: they keep the PE activity
    monitor from re-throttling the clock to 1.2GHz after every
    collective (the re-warm otherwise costs ~30us at half clock).
  - BN rstd uses DVE reciprocal + ACT sqrt; the sqrt<->exp table-set
    reloads hide behind the stats landing DMA / attention score matmuls.

Numerics: all matmuls in bf16 (fp32 PSUM accumulation); x kept in fp32
master + bf16 matmul copy; BN statistics in fp32.

Layout: activations transposed [128 part, dtile, tokens]; weights natural
[Din, Dout] serve as lhsT. Attention per head pair packs the two heads at
partition bases 0/64 (row-group concurrency on the PE). Softmax sums ride
as a ones-column in the even head's V (PSUM rows 64) and a ones-row matmul
into PSUM row 96 for the odd head. Embedding gather uses
dma_gather(transpose=True), which lands rows directly in the transposed
layout.
"""

import os

import numpy as np

import concourse.bass as bass
import concourse.mybir as mybir
import concourse.tile as tile
from concourse import bacc
from concourse.bass import ts

# ---------------------------------------------------------------- dims
V, D, L, H, B, S = 32000, 1024, 6, 16, 4, 512
HD = D // H            # 64
DF = 4 * D             # 4096
EPS = 1e-3
NC = 8                 # cores
T = B * S              # 2048 tokens total
TL = T // NC           # 256 tokens per core
P = 128                # partitions
DT = D // P            # 8 d-tiles
FMT = DF // P          # 32 ffn1 m-tiles
SK = S // P            # 4 key chunks per batch

f32 = mybir.dt.float32
bf16 = mybir.dt.bfloat16
i16 = mybir.dt.int16
AF = mybir.ActivationFunctionType
ALU = mybir.AluOpType

ALLGRP = [list(range(NC))]
KVGRP = [[2 * b, 2 * b + 1] for b in range(B)]

N_LAYERS = int(os.environ.get("TRN_KERNEL_LAYERS", str(L)))
DEBUG_TAPS = os.environ.get("TRN_KERNEL_DEBUG", "0") == "1"
DBG_LAYER = int(os.environ.get("TRN_DBG_LAYER", "0"))

XB = D * TL            # elements of one rank's xb16 block in the x exchange


def build_module(n_layers=None):
    if n_layers is None:
        n_layers = N_LAYERS
    nc = bacc.Bacc("TRN2", target_bir_lowering=False, debug=False,
                   num_devices=NC)

    dt_ = nc.dram_tensor
    io = {
        "emb": dt_("emb", [V, D], bf16, kind="ExternalInput").ap(),
        "idx": dt_("idx", [16, TL // 16], i16, kind="ExternalInput").ap(),
        "pesT": dt_("pesT", [D, TL], f32, kind="ExternalInput").ap(),
        "wq": dt_("wq", [L, D, D], bf16, kind="ExternalInput").ap(),
        "wk": dt_("wk", [L, D, D], bf16, kind="ExternalInput").ap(),
        "wv": dt_("wv", [L, D, D], bf16, kind="ExternalInput").ap(),
        "wo": dt_("wo", [L, D, D], bf16, kind="ExternalInput").ap(),
        "w1": dt_("w1", [L, D, DF], bf16, kind="ExternalInput").ap(),
        "w2": dt_("w2", [L, DF, D], bf16, kind="ExternalInput").ap(),
        "bq": dt_("bq", [L, P, DT], f32, kind="ExternalInput").ap(),
        "b1": dt_("b1", [L, P, FMT], f32, kind="ExternalInput").ap(),
        "g1": dt_("g1", [L, P, DT], f32, kind="ExternalInput").ap(),
        "be1": dt_("be1", [L, P, DT], f32, kind="ExternalInput").ap(),
        "g2": dt_("g2", [L, P, DT], f32, kind="ExternalInput").ap(),
        "be2": dt_("be2", [L, P, DT], f32, kind="ExternalInput").ap(),
        "out": dt_("out", [D, TL], f32, kind="ExternalOutput").ap(),
    }
    if DEBUG_TAPS:
        for nm, shp, dt in [
                ("dbg_xext", [D, S], bf16), ("dbg_q", [D, TL], bf16),
                ("dbg_k", [D, S], bf16),
                ("dbg_v", [P, SK * H * (HD + 1)], bf16),
                ("dbg_attnT", [D, TL], bf16), ("dbg_y1", [D, TL], f32),
                ("dbg_x2", [D, TL], f32), ("dbg_h", [DF, TL], bf16),
                ("dbg_y2", [D, TL], f32)]:
            io[nm] = dt_(nm, shp, dt, kind="ExternalOutput").ap()

    with tile.TileContext(nc) as tc:
        _build(tc, n_layers, io)
    nc.compile()
    return nc


def _build(tc, n_layers, io):
    from contextlib import ExitStack
    nc = tc.nc
    att_scale = 1.0 / np.sqrt(HD)
    dmae = nc.scalar

    st_ = ExitStack()
    persist = st_.enter_context(tc.tile_pool(name="persist", bufs=1))
    wpool = st_.enter_context(tc.tile_pool(name="wpool", bufs=10))
    ppool = st_.enter_context(tc.tile_pool(name="ppool", bufs=2))
    epool = st_.enter_context(tc.tile_pool(name="epool", bufs=10))
    spool = st_.enter_context(tc.tile_pool(name="spool", bufs=2))
    ps = st_.enter_context(tc.tile_pool(name="ps", bufs=4, space="PSUM"))
    drin = st_.enter_context(tc.tile_pool(name="drin", bufs=2, space="DRAM"))
    drout = st_.enter_context(tc.tile_pool(name="drout", bufs=2, space="DRAM"))

    # ---------------- persistent tiles
    xf32a = persist.tile([P, DT, TL], f32, name="xf32a")
    xf32b = persist.tile([P, DT, TL], f32, name="xf32b")
    xb16 = persist.tile([P, DT, TL], bf16, name="xb16")
    xext = persist.tile([P, DT, S], bf16, name="xext")
    qT = persist.tile([P, DT, TL], bf16, name="qT")
    kT = persist.tile([P, DT, S], bf16, name="kT")
    vsb = persist.tile([P, SK, H, HD + 1], bf16, name="vsb")
    attnT = persist.tile([P, DT, TL], bf16, name="attnT")
    ht = persist.tile([P, FMT, TL], bf16, name="ht")
    y2b16 = persist.tile([P, DT, TL], bf16, name="y2b16")
    onesb = persist.tile([P, P], bf16, name="onesb")
    dmx = persist.tile([P, 512], bf16, name="dmx")
    dscr = persist.tile([P, 1], bf16, name="dscr")
    epsb = persist.tile([P, 1], f32, name="epsb")
    idxs = persist.tile([P, TL // 16], i16, name="idxs")

    nc.vector.memset(onesb[:], 1.0)
    nc.vector.memset(dmx[:], 1.0)
    nc.vector.memset(epsb[:], EPS)
    nc.vector.memset(vsb[:, :, :, HD:HD + 1], 1.0)
    for r_ in range(P // 16):
        nc.sync.dma_start(idxs[16 * r_:16 * (r_ + 1), :], io["idx"])

    # weight chunk loader: returns [P, DT, 512] (half the out-cols of a DxD
    # weight) or [P, 4, D] (4 k-tiles of w2). Q/K/V chunks ride the gpsimd
    # engine's DMA queue so they never sit behind the 16MB of W1/W2 traffic
    # on the sync queue (Q proj must start the moment BN2's xb16 lands).
    def wchunk(src_ap, nm, eng=None):
        t = wpool.tile(list(src_ap.shape), bf16, tag="w", name=nm)
        (eng or nc.sync).dma_start(t[:], src_ap)
        return t

    def load_kvq(l):
        wq_r = io["wq"][l].rearrange("(k p) m -> p k m", p=P)
        wk_r = io["wk"][l].rearrange("(k p) m -> p k m", p=P)
        wv_r = io["wv"][l].rearrange("(k p) m -> p k m", p=P)
        wq_ch = [wchunk(wq_r[:, :, ts(h, 512)], f"wq{l}_{h}", nc.gpsimd)
                 for h in range(2)]
        wk_ch = [wchunk(wk_r[:, :, ts(h, 512)], f"wk{l}_{h}", nc.gpsimd)
                 for h in range(2)]
        wv_ch = [wchunk(wv_r[:, :, ts(h, 512)], f"wv{l}_{h}", nc.gpsimd)
                 for h in range(2)]
        return wq_ch, wk_ch, wv_ch

    # keep-warm fillers: dense dummy matmuls issued into a sync bubble so
    # the PE activity monitor doesn't re-throttle the clock. They read only
    # never-rewritten tiles (onesb/dmx) so they create no dependencies and
    # start the moment the preceding real matmul retires.
    NFILL = int(os.environ.get("TRN_FILL", "24"))
    NFILLG = int(os.environ.get("TRN_FILLG", "0"))

    def pe_fill(nm, anchor, n=None):
        n = NFILL if n is None else n
        if n == 0:
            return
        scr = ps.tile([P, 512], f32, tag="u", bufs=2, name=f"fill{nm}")
        nc.tensor.matmul(scr[:], onesb[:], anchor, start=True, stop=True)
        for _ in range(n - 1):
            nc.tensor.matmul(scr[:], onesb[:], dmx[:], start=True, stop=True)
        with nc.allow_low_precision(reason="keep-warm filler sink"):
            nc.vector.tensor_copy(dscr[:, 0:1], scr[:, 0:1])

    # pair AllGather of src (xb16 at the embedding; raw y2 per layer): both
    # halves land in xext in fixed rank-major order. Attention is key-order
    # invariant, so no rank-dependent indexing is needed anywhere.
    def x_ship(lbl, src):
        xi = drin.tile([XB], bf16, tag="xi", name=f"xi{lbl}")
        xo = drout.tile([2 * XB], bf16, tag="xo", name=f"xo{lbl}")
        dmae.dma_start(
            xi[:].rearrange("(g p t) -> p g t", g=DT, p=P), src[:])
        nc.gpsimd.collective_compute(
            "AllGather", ALU.bypass, replica_groups=KVGRP,
            ins=[xi[:].opt()], outs=[xo[:].opt()])
        return xo

    def x_land(xo, sc=None, sh=None):
        for hf in range(2):
            dmae.dma_start(
                xext[:, :, ts(hf, TL)],
                xo[hf * XB:(hf + 1) * XB].rearrange(
                    "(g p t) -> p g t", g=DT, p=P))
        if sc is not None:
            for m in range(DT):
                nc.vector.tensor_scalar(out=xext[:, m, :], in0=xext[:, m, :],
                                        scalar1=sc[:, m:m + 1],
                                        scalar2=sh[:, m:m + 1],
                                        op0=ALU.mult, op1=ALU.add)

    # ---------------- embedding: x^T = (emb[seq])^T + pes^T
    # (gather first: layer-0's kvq prefetch shares the gpsimd DMA queue and
    # must not delay the critical path to xb16 / the first x exchange)
    pes_sb = spool.tile([P, DT, TL], f32, tag="pes", bufs=1, name="pes_sb")
    dmae.dma_start(pes_sb[:], io["pesT"].rearrange("(k p) t -> p k t", p=P))
    gt = spool.tile([P, DT, TL], bf16, tag="gt", bufs=1, name="gt")
    nc.gpsimd.dma_gather(
        out_ap=gt[:], in_ap=io["emb"], idxs_ap=idxs[:],
        num_idxs=TL, num_idxs_reg=TL, elem_size=D, transpose=True)
    kvq_next = load_kvq(0)
    for k in range(DT):
        nc.vector.tensor_tensor(out=xf32a[:, k, :], in0=gt[:, k, :],
                                in1=pes_sb[:, k, :], op=ALU.add)
    nc.vector.tensor_copy(xb16[:], xf32a[:])
    x_land(x_ship("e", xb16))

    xcur = xf32a
    xnxt = xf32b

    # ---------------- per-layer param loads (small)
    def load_params(l):
        bq_sb = ppool.tile([P, DT], f32, tag="bq", name=f"bq{l}")
        b1_sb = ppool.tile([P, FMT], f32, tag="b1", name=f"b1{l}")
        g1_sb = ppool.tile([P, DT], f32, tag="g1", name=f"g1{l}")
        be1_sb = ppool.tile([P, DT], f32, tag="be1", name=f"be1{l}")
        g2_sb = ppool.tile([P, DT], f32, tag="g2", name=f"g2{l}")
        be2_sb = ppool.tile([P, DT], f32, tag="be2", name=f"be2{l}")
        dmae.dma_start(bq_sb[:], io["bq"][l])
        dmae.dma_start(b1_sb[:], io["b1"][l])
        dmae.dma_start(g1_sb[:], io["g1"][l])
        dmae.dma_start(be1_sb[:], io["be1"][l])
        dmae.dma_start(g2_sb[:], io["g2"][l])
        dmae.dma_start(be2_sb[:], io["be2"][l])
        return bq_sb, b1_sb, g1_sb, be1_sb, g2_sb, be2_sb

    # BN stats -> AllGather -> sc/sh. rstd = exp(-0.5*ln(var+eps)) keeps the
    # ACT engine on the natural_log_exp table set for the whole kernel (the
    # Sqrt set would force a ~2.7us table reload twice per layer).
    def bn_reduce(lbl, stats, g_sb, be_sb):
        sti = drin.tile([P * 16], f32, tag="sti", name=f"sti{lbl}")
        sto = drout.tile([NC * P * 16], f32, tag="sto", addr_space="Shared",
                         name=f"sto{lbl}")
        dmae.dma_start(sti[:].rearrange("(p s) -> p s", p=P), stats[:])
        nc.gpsimd.collective_compute(
            "AllGather", ALU.bypass, replica_groups=ALLGRP,
            ins=[sti[:].opt()], outs=[sto[:].opt()])
        # land rank-major (16 contiguous fp32 per descriptor segment; an
        # s-innermost layout would degrade the DMA to 4-byte elements and
        # cost ~20us), then tree-reduce the 8 rank blocks on the DVE.
        ld = spool.tile([P, NC, 16], f32, tag="ld", name=f"ld{lbl}")
        dmae.dma_start(ld[:], sto[:].rearrange("(r p s) -> p r s", p=P, s=16))
        u1 = spool.tile([P, 4, 16], f32, tag="u1", name=f"u1{lbl}")
        nc.vector.tensor_tensor(out=u1[:], in0=ld[:, 0:4, :], in1=ld[:, 4:8, :],
                                op=ALU.add)
        u2 = spool.tile([P, 2, 16], f32, tag="u2", name=f"u2{lbl}")
        nc.vector.tensor_tensor(out=u2[:], in0=u1[:, 0:2, :], in1=u1[:, 2:4, :],
                                op=ALU.add)
        tot = spool.tile([P, 16], f32, tag="tot", name=f"tot{lbl}")
        nc.vector.tensor_tensor(out=tot[:], in0=u2[:, 0, :], in1=u2[:, 1, :],
                                op=ALU.add)
        mean = spool.tile([P, DT], f32, tag="mean", name=f"mean{lbl}")
        nc.vector.tensor_scalar_mul(mean[:], tot[:, 0:DT], 1.0 / T)
        msq = spool.tile([P, DT], f32, tag="msq", name=f"msq{lbl}")
        nc.vector.tensor_tensor(out=msq[:], in0=mean[:], in1=mean[:], op=ALU.mult)
        veps = spool.tile([P, DT], f32, tag="veps", name=f"veps{lbl}")
        nc.vector.scalar_tensor_tensor(out=veps[:], in0=tot[:, DT:16],
                                       scalar=1.0 / T, in1=msq[:],
                                       op0=ALU.mult, op1=ALU.subtract)
        if os.environ.get("TRN_RSTD_SQRT", "1") == "1":
            nc.vector.tensor_scalar_add(veps[:], veps[:], EPS)
            rec = spool.tile([P, DT], f32, tag="lnv", name=f"rec{lbl}")
            nc.vector.reciprocal(rec[:], veps[:])
            rstd = spool.tile([P, DT], f32, tag="rstd", name=f"rstd{lbl}")
            nc.scalar.sqrt(rstd[:], rec[:])
        else:
            lnv = spool.tile([P, DT], f32, tag="lnv", name=f"lnv{lbl}")
            nc.scalar.activation(lnv[:], veps[:], AF.Ln, bias=epsb[:, 0:1])
            rstd = spool.tile([P, DT], f32, tag="rstd", name=f"rstd{lbl}")
            nc.scalar.activation(rstd[:], lnv[:], AF.Exp, scale=-0.5)
        sc = spool.tile([P, DT], f32, tag="sc", name=f"sc{lbl}")
        nc.vector.tensor_tensor(out=sc[:], in0=g_sb[:], in1=rstd[:], op=ALU.mult)
        sh = spool.tile([P, DT], f32, tag="sh", name=f"sh{lbl}")
        nc.vector.tensor_tensor(out=sh[:], in0=mean[:], in1=sc[:], op=ALU.mult)
        nc.vector.tensor_tensor(out=sh[:], in0=be_sb[:], in1=sh[:], op=ALU.subtract)
        return sc, sh

    # ---------------- layers
    for l in range(n_layers):
        bq_sb, b1_sb, g1_sb, be1_sb, g2_sb, be2_sb = load_params(l)
        wq_ch, wk_ch, wv_ch = kvq_next

        wo_r = io["wo"][l].rearrange("(k p) m -> p k m", p=P)
        w1_r = io["w1"][l].rearrange("(k p) m -> p k m", p=P)
        w2_r = io["w2"][l].rearrange("(k p) m -> p k m", p=P)

        # ---- Q projection (local tokens; overlaps the x AllGather)
        for g in range(DT):
            psq = ps.tile([P, TL], f32, tag="mm", name=f"psq{l}_{g}")
            for k in range(DT):
                nc.tensor.matmul(psq[:], wq_ch[g // 4][:, k, ts(g % 4, P)],
                                 xb16[:, k, :], start=(k == 0), stop=(k == DT - 1))
            nc.vector.tensor_scalar_add(qT[:, g, :], psq[:], bq_sb[:, g:g + 1])

        # keep the clock warm across the x-AllGather tail (K/V proj below
        # block on the xext landing; Q proj above is the only real cover)
        pe_fill(f"q{l}", qT[:, 6:8, :], n=16)

        # ---- K projection over the full 512-token batch sequence
        for g in range(DT):
            psk = ps.tile([P, S], f32, tag="mm", name=f"psk{l}_{g}")
            for k in range(DT):
                nc.tensor.matmul(psk[:], wk_ch[g // 4][:, k, ts(g % 4, P)],
                                 xext[:, k, :], start=(k == 0), stop=(k == DT - 1))
            nc.vector.tensor_copy(kT[:, g, :], psk[:])

        # ---- V projection, token-major: V = x W_v (xext tiles stationary)
        for mt in range(SK):
            for nb in range(4):
                psv = ps.tile([P, TL], f32, tag="mm", name=f"psv{l}_{mt}_{nb}")
                for k in range(DT):
                    nc.tensor.matmul(
                        psv[:], xext[:, k, ts(mt, P)],
                        wv_ch[nb // 2][:, k, ts(nb % 2, 256)],
                        start=(k == 0), stop=(k == DT - 1))
                nc.vector.tensor_copy(
                    vsb[:, mt, 4 * nb:4 * nb + 4, 0:HD],
                    psv[:].rearrange("p (h x) -> p h x", h=4))

        if DEBUG_TAPS and l == DBG_LAYER:
            nc.sync.dma_start(io["dbg_xext"].rearrange("(k p) t -> p k t", p=P),
                              xext[:])
            nc.sync.dma_start(io["dbg_q"].rearrange("(k p) t -> p k t", p=P),
                              qT[:])
            nc.sync.dma_start(io["dbg_k"].rearrange("(k p) t -> p k t", p=P),
                              kT[:])
            nc.sync.dma_start(io["dbg_v"],
                              vsb[:].rearrange("p a h x -> p (a h x)"))

        wo_ch = [wchunk(wo_r[:, :, ts(h, 512)], f"wo{l}_{h}") for h in range(2)]

        # ---- attention phase A: scores + exp (batched over 2 key-chunks)
        eall = []
        for g in range(DT):
            epair = [[None, None], [None, None]]
            for kcb in range(2):
                for tw in range(2):
                    hp = 64 * tw
                    sst = ps.tile([P, 2, TL], f32, tag="s", bufs=2,
                                  name=f"pss{l}_{g}_{kcb}_{tw}")
                    for j in range(2):
                        nc.tensor.matmul(
                            sst[:, j, :],
                            kT[hp:hp + HD, g, ts(2 * kcb + j, P)],
                            qT[hp:hp + HD, g, :], start=True, stop=True)
                    et = epool.tile([P, 2, TL], bf16, tag="e", bufs=16,
                                    name=f"et{l}_{g}_{kcb}_{tw}")
                    nc.scalar.activation(et[:], sst[:], AF.Exp, scale=att_scale)
                    epair[tw][kcb] = et
            eall.append(epair)

        # ---- phase B + k-outer Wo: per pair g accumulate U over the 4 key
        # chunks, normalize, then feed the Wo accumulators with attnT[:,g,:]
        # to keep the PE dense through the ACT-bound softmax phase.
        wo_acc = [ps.tile([P, 2, TL], f32, tag="mm", name=f"woa{l}_{m}")
                  for m in range(4)]
        st1 = spool.tile([P, 16], f32, tag="st", name=f"st1_{l}")
        sqs = spool.tile([P, TL], f32, tag="sqs", name=f"sq1_{l}")
        for g in range(DT):
            epair = eall[g]
            bankA = ps.tile([P, TL], f32, tag="u", bufs=2, name=f"bA{l}_{g}")
            bankB = ps.tile([P, TL], f32, tag="u", bufs=2, name=f"bB{l}_{g}")
            for kc in range(SK):
                fl, ll = (kc == 0), (kc == SK - 1)
                ee = epair[0][kc // 2][:, kc % 2, :]
                eo = epair[1][kc // 2][:, kc % 2, :]
                nc.tensor.matmul(bankA[0:HD + 1, :],
                                 vsb[:, kc, 2 * g, 0:HD + 1], ee,
                                 start=fl, stop=ll)
                nc.tensor.matmul(bankB[64:128, :],
                                 vsb[:, kc, 2 * g + 1, 0:HD], eo,
                                 start=fl, stop=ll)
                nc.tensor.matmul(bankA[96:97, :], onesb[:, 0:1], eo,
                                 start=fl, stop=ll, tile_position=(0, 96))
            rs = spool.tile([P, TL], bf16, tag="rs", name=f"rs{l}_{g}")
            with nc.allow_low_precision(reason="softmax 1/sumexp as bf16"):
                nc.vector.reciprocal(rs[HD:97, :], bankA[HD:97, :])
            # broadcast the reciprocals into the dead halves of the U banks
            nc.tensor.matmul(bankB[0:64, :], onesb[HD:HD + 1, 0:64],
                             rs[HD:HD + 1, :], start=True, stop=True,
                             tile_position=(64, 0))
            nc.tensor.matmul(bankA[64:128, :], onesb[96:97, 0:64],
                             rs[96:97, :], start=True, stop=True,
                             tile_position=(96, 64))
            usbE = epool.tile([P, TL], bf16, tag="usb", bufs=6,
                              name=f"uE{l}_{g}")
            usbO = epool.tile([P, TL], bf16, tag="usb", bufs=6,
                              name=f"uO{l}_{g}")
            nc.vector.tensor_copy(usbE[0:64, :], bankA[0:64, :])
            nc.vector.tensor_copy(usbO[64:128, :], bankB[64:128, :])
            nc.vector.tensor_tensor(out=attnT[0:64, g, :], in0=usbE[0:64, :],
                                    in1=bankB[0:64, :], op=ALU.mult)
            nc.vector.tensor_tensor(out=attnT[64:128, g, :], in0=usbO[64:128, :],
                                    in1=bankA[64:128, :], op=ALU.mult)
            if NFILLG and g < DT - 1:
                # keep the PE activity monitor fed through the exp-bound
                # stretch between head pairs (attention otherwise cools the
                # clock to 1.2GHz for the rest of the phase)
                scrg = ps.tile([P, 512], f32, tag="u", bufs=2,
                               name=f"fillg{l}_{g}")
                nc.tensor.matmul(scrg[:, 0:TL], onesb[:], attnT[:, g, :],
                                 start=True, stop=True)
                for _ in range(NFILLG - 1):
                    nc.tensor.matmul(scrg[:], onesb[:], dmx[:],
                                     start=True, stop=True)
                with nc.allow_low_precision(reason="keep-warm filler sink"):
                    nc.vector.tensor_copy(dscr[:, 0:1], scrg[:, 0:1])

        for m in range(DT):
            for g in range(DT):
                nc.tensor.matmul(wo_acc[m // 2][:, m % 2, :],
                                 wo_ch[m // 4][:, g, ts(m % 4, P)],
                                 attnT[:, g, :],
                                 start=(g == 0), stop=(g == DT - 1))

        w1_ch = [wchunk(w1_r[:, :, ts(h, 512)], f"w1{l}_{h}") for h in range(8)]

        # ---- residual -> y1 (fp32) with fused BN partial stats
        for m in range(DT):
            nc.vector.scalar_tensor_tensor(
                out=xnxt[:, m, :], in0=wo_acc[m // 2][:, m % 2, :], scalar=1.0,
                in1=xcur[:, m, :], op0=ALU.mult, op1=ALU.add,
                accum_out=st1[:, m:m + 1])
            nc.scalar.activation(sqs[:], xnxt[:, m, :], AF.Square,
                                 accum_out=st1[:, DT + m:DT + m + 1])

        if DEBUG_TAPS and l == DBG_LAYER:
            nc.sync.dma_start(io["dbg_attnT"].rearrange("(k p) t -> p k t", p=P),
                              attnT[:])
            nc.sync.dma_start(io["dbg_y1"].rearrange("(k p) t -> p k t", p=P),
                              xnxt[:])

        pe_fill(f"a{l}", attnT[:, 6:8, :])
        sc1, sh1 = bn_reduce(f"a{l}", st1, g1_sb, be1_sb)
        for m in range(DT):
            nc.scalar.activation(xb16[:, m, :], xnxt[:, m, :], AF.Identity,
                                 bias=sh1[:, m:m + 1], scale=sc1[:, m:m + 1])
            nc.vector.tensor_scalar(out=xnxt[:, m, :], in0=xnxt[:, m, :],
                                    scalar1=sc1[:, m:m + 1],
                                    scalar2=sh1[:, m:m + 1],
                                    op0=ALU.mult, op1=ALU.add)
        xcur, xnxt = xnxt, xcur

        if DEBUG_TAPS and l == DBG_LAYER:
            nc.sync.dma_start(io["dbg_x2"].rearrange("(k p) t -> p k t", p=P),
                              xcur[:])

        w2_ch = [wchunk(w2_r[:, ts(h, 4), :], f"w2{l}_{h}") for h in range(8)]

        # ---- FFN1: h^T = relu(W1^T x^T + b1)
        for m in range(FMT):
            ps1 = ps.tile([P, TL], f32, tag="mm", name=f"ps1{l}_{m}")
            for k in range(DT):
                nc.tensor.matmul(ps1[:], w1_ch[m // 4][:, k, ts(m % 4, P)],
                                 xb16[:, k, :], start=(k == 0), stop=(k == DT - 1))
            nc.scalar.activation(ht[:, m, :], ps1[:], AF.Relu,
                                 bias=b1_sb[:, m:m + 1])

        # ---- FFN2 + residual -> y2 with fused BN partial stats
        st2 = spool.tile([P, 16], f32, tag="st", name=f"st2_{l}")
        sqs2 = spool.tile([P, TL], f32, tag="sqs", name=f"sq2_{l}")
        HF = FMT // 2
        for m in range(DT):
            ps2 = ps.tile([P, TL], f32, tag="mm", name=f"ps2a{l}_{m}")
            for k in range(HF):
                nc.tensor.matmul(ps2[:], w2_ch[k // 4][:, k % 4, ts(m, P)],
                                 ht[:, k, :], start=(k == 0), stop=(k == HF - 1))
            nc.vector.scalar_tensor_tensor(
                out=xnxt[:, m, :], in0=ps2[:], scalar=1.0, in1=xcur[:, m, :],
                op0=ALU.mult, op1=ALU.add)
        for m in range(DT):
            ps2 = ps.tile([P, TL], f32, tag="mm", name=f"ps2b{l}_{m}")
            for k in range(HF, FMT):
                nc.tensor.matmul(ps2[:], w2_ch[k // 4][:, k % 4, ts(m, P)],
                                 ht[:, k, :], start=(k == HF), stop=(k == FMT - 1))
            nc.vector.scalar_tensor_tensor(
                out=xnxt[:, m, :], in0=ps2[:], scalar=1.0, in1=xnxt[:, m, :],
                op0=ALU.mult, op1=ALU.add, accum_out=st2[:, m:m + 1])
            nc.scalar.activation(sqs2[:], xnxt[:, m, :], AF.Square,
                                 accum_out=st2[:, DT + m:DT + m + 1])
            if l + 1 < n_layers:
                nc.scalar.copy(y2b16[:, m, :], xnxt[:, m, :])

        if DEBUG_TAPS and l == DBG_LAYER:
            nc.sync.dma_start(io["dbg_h"].rearrange("(k p) t -> p k t", p=P),
                              ht[:])
            nc.sync.dma_start(io["dbg_y2"].rearrange("(k p) t -> p k t", p=P),
                              xnxt[:])

        if l + 1 < n_layers:
            pe_fill(f"f{l}", y2b16[:, 6:8, :])
        sc2, sh2 = bn_reduce(f"f{l}", st2, g2_sb, be2_sb)
        if l + 1 < n_layers:
            xo_next = x_ship(f"x{l + 1}", y2b16)
            x_land(xo_next, sc2, sh2)
            # prefetch next layer's Q/K/V weights into the BN2 bubble (the
            # gpsimd DMA queue is otherwise idle here, and this keeps their
            # 6MB off the HBM while FFN2 streams W2)
            kvq_next = load_kvq(l + 1)
        for m in range(DT):
            if l + 1 < n_layers:
                nc.scalar.activation(xb16[:, m, :], xnxt[:, m, :], AF.Identity,
                                     bias=sh2[:, m:m + 1], scale=sc2[:, m:m + 1])
            nc.vector.tensor_scalar(out=xnxt[:, m, :], in0=xnxt[:, m, :],
                                    scalar1=sc2[:, m:m + 1],
                                    scalar2=sh2[:, m:m + 1],
                                    op0=ALU.mult, op1=ALU.add)
        xcur, xnxt = xnxt, xcur

    # ---------------- output x^T local slice (per tile, overlapping the
    # final BN apply chain)
    outr = io["out"].rearrange("(k p) t -> p k t", p=P)
    for m in range(DT):
        dmae.dma_start(outr[:, m, :], xcur[:, m, :])
    st_.close()


# ================================================================ host side

def make_in_maps(inputs):
    import ml_dtypes
    bf = lambda a: np.ascontiguousarray(np.asarray(a, dtype=np.float32)).astype(
        ml_dtypes.bfloat16)
    f = lambda a: np.ascontiguousarray(np.asarray(a), dtype=np.float32)
    seq = np.asarray(inputs["sequence"]).reshape(-1).astype(np.int16)
    emb = bf(inputs["emb"])
    pesT = np.ascontiguousarray(f(inputs["pes"]).T)            # [D, S]
    wq, wk, wv = bf(inputs["Wq"]), bf(inputs["Wk"]), bf(inputs["Wv"])
    wo, w1, w2 = bf(inputs["Wo"]), bf(inputs["W1"]), bf(inputs["W2"])
    pt = lambda a, m: np.ascontiguousarray(
        f(a).reshape(L, m, P).transpose(0, 2, 1))   # [L, P, m] with ch = m*128+p
    bq, b1 = pt(inputs["bq"], DT), pt(inputs["b1"], FMT)
    g1, be1 = pt(inputs["g1"], DT), pt(inputs["be1"], DT)
    g2, be2 = pt(inputs["g2"], DT), pt(inputs["be2"], DT)

    in_maps = []
    for c in range(NC):
        loc = seq[c * TL:(c + 1) * TL]
        idx = np.ascontiguousarray(loc.reshape(TL // 16, 16).T)    # [16, TL/16]
        off = (c % 2) * TL
        in_maps.append({
            "emb": emb,
            "idx": idx,
            "pesT": np.ascontiguousarray(pesT[:, off:off + TL]),
            "wq": wq, "wk": wk, "wv": wv, "wo": wo, "w1": w1, "w2": w2,
            "bq": bq, "b1": b1,
            "g1": g1, "be1": be1, "g2": g2, "be2": be2,
        })
    return in_maps


_CACHE = {}


def _get_module():
    if "nc" not in _CACHE:
        _CACHE["nc"] = build_module()
    return _CACHE["nc"]


def kernel(**inputs):
    from concourse import bass_utils
    nc = _get_module()
    in_maps = make_in_maps(inputs)
    res = bass_utils.run_bass_kernel_spmd(nc, in_maps, list(range(NC)))
    full = np.concatenate(
        [np.asarray(res.results[c]["out"]) for c in range(NC)], axis=1)
    return np.ascontiguousarray(full.T).reshape(B, S, D).astype(np.float32)
